# revision 18
# baseline (speedup 1.0000x reference)
"""Trainium2 Bass kernel for nn_Net_18906446037087 (snntorch Leaky SNN layer).

Reference semantics (per batch element, 255 steps, f32):
    cur = x @ W.T                         # [B, 1]
    m_0 = 0
    m_{t+1} = (0.95*m_t + cur) * (m_t <= 1)
    spk_{t+1} = (m_{t+1} > 1)
Outputs: (spk_rec, mem_rec), each [255, B, 1] f32.

Sharding: pure data parallel over batch across 8 cores (B=65536 -> 8192/core).

Closed form: the trajectory is periodic in t. With s[k] = (1-b^k)/(1-b),
an element first spikes at step K iff cur > 1/s[K]; then mem repeats the
pattern A_K[t] = s[((t-1) mod (K+1)) + 1] (0 at the reset slot); elements
with cur <= 1/s[255] follow the pure ramp R[t] = s[t]. So
    mem[t, b] = cur_b * (R[t] + sum_{k>=K(b)} (A_k - A_{k+1})[t])
which is one matmul  mem = G^T @ F  with
    G[0] = R, G[k] = A_k - A_{k+1} (A_256 := R)     (host-precomputed)
    F[k, b] = cur_b * [cur_b > theta_k], theta_0 = -inf, theta_k = 1/s[k].
spk is derived on host as mem > 1.0 (exact).

Numerics: the matvec and the G@F matmul run in fp32r (PE rounds operands
to 11 explicit mantissa bits, RNE — measured on HW); end-to-end rel err
~2e-3 vs the 2e-2 gate (validated against the axon-backend oracle).

Per-core pipeline (B_CORE=8192 = 16 groups of 512 = 64 subgroups of 128):
  per group g:
   - one DMA loads x rows as [128, 4, 784] (subgroup-major, contiguous);
   - PE transposes 7 K-chunks (fp32r, 1.5 cyc/row) into PSUM; DVE/ACT
     evacuate to SBUF; 28 matmuls with xT *stationary* and W moving
     (out [128,1]/subgroup, PSUM bank pre-zeroed + start=False because a
     start=True matmul resets its whole PSUM bank) -> cur columns;
   - cur [128,4] is PE-transposed to rows and bounced 2KB through DRAM
     to a [1,512] row (ACT-issued DMAs);
   - one group later (hiding the bounce), PE broadcasts the row to
     [128,512] via a ones outer product, DVE stt builds F (2 class
     chunks, fp32r), two accumulated fp32r matmuls per 128-step slab
     produce mem[t,b] in [t-partition, batch-free] layout, DVE/ACT
     evacuate, ACT issues contiguous 2KB-per-partition writes.
DMA issue queues: x loads alone on SP (so nothing blocks them); all
tail DMAs on ACT, each emitted right after its producing copy.

TimelineSim budget per core: DMA ~96us busy (floor: x in 71.4 + mem out
23.2), PE ~65, DVE ~75, ACT ~55.
"""
import sys
if "/opt/trn_rl_repo" not in sys.path:
    sys.path.insert(0, "/opt/trn_rl_repo")

import numpy as np
from contextlib import ExitStack

import concourse.bass as bass
import concourse.bacc as bacc
import concourse.mybir as mybir
import concourse.tile as tile
from concourse.bass_utils import run_bass_kernel_spmd

F32 = mybir.dt.float32
F32R = mybir.dt.float32r
ALU = mybir.AluOpType

N_CORES = 8
B_FULL = 65536
B_CORE = B_FULL // N_CORES          # 8192
D = 784
NUM_STEPS = 255
BETA = 0.95
THRESHOLD = 1.0

GROUP = 512                          # batch per group
NGROUP = B_CORE // GROUP             # 16
CHUNKS = [(0, 128), (128, 128), (256, 128), (384, 128), (512, 128), (640, 128), (768, 16)]
NCLASS = 256                         # class 0 = ramp; class k = first spike at k
TCHUNKS = [(0, 128), (128, 127)]     # step slabs (255 rows)


def _build():
    nc = bacc.Bacc("TRN2", target_bir_lowering=False, debug=False,
                   num_devices=N_CORES)
    x_d = nc.dram_tensor("x", [B_CORE, D], F32R, kind="ExternalInput")
    w_d = nc.dram_tensor("w", [128, 7], F32, kind="ExternalInput")
    id_d = nc.dram_tensor("ident", [128, 128], F32R, kind="ExternalInput")
    g_d = nc.dram_tensor("gtab", [128, 2 * NUM_STEPS], F32R, kind="ExternalInput")
    thr_d = nc.dram_tensor("thr", [128, 2], F32, kind="ExternalInput")
    ones_d = nc.dram_tensor("ones", [1, 128], F32, kind="ExternalInput")
    mem_d = nc.dram_tensor("mem", [NUM_STEPS, B_CORE], F32, kind="ExternalOutput")

    with tile.TileContext(nc) as tc, ExitStack() as ctx:
        xpool = ctx.enter_context(tc.tile_pool(name="xpool", bufs=5))
        xtpool = ctx.enter_context(tc.tile_pool(name="xtpool", bufs=4))
        rowpool = ctx.enter_context(tc.tile_pool(name="rowpool", bufs=2))
        fpool = ctx.enter_context(tc.tile_pool(name="fpool", bufs=2))
        opool = ctx.enter_context(tc.tile_pool(name="opool", bufs=4))
        const = ctx.enter_context(tc.tile_pool(name="const", bufs=1))
        psxt = ctx.enter_context(tc.tile_pool(name="psxt", bufs=3, space="PSUM"))
        psacc = ctx.enter_context(tc.tile_pool(name="psacc", bufs=1, space="PSUM"))
        psct = ctx.enter_context(tc.tile_pool(name="psct", bufs=1, space="PSUM"))
        psbc = ctx.enter_context(tc.tile_pool(name="psbc", bufs=1, space="PSUM"))
        psgo = ctx.enter_context(tc.tile_pool(name="psgo", bufs=2, space="PSUM"))

        w_t = const.tile([128, 7], F32)
        id_t = const.tile([128, 128], F32R)
        g_t = const.tile([128, 2 * NUM_STEPS], F32R)
        thr_t = const.tile([128, 2], F32)
        ones_t = const.tile([1, 128], F32)

        cur_cols = const.tile([128, NGROUP * 4], F32, name="cur_cols")

        copy_engines = [nc.vector.tensor_copy, nc.scalar.copy]
        copy_idx = [0]

        def copy(out, in_):
            eng = copy_engines[copy_idx[0] % len(copy_engines)]
            copy_idx[0] += 1
            eng(out, in_)

        # x rows for group g as [128, 4, D]: partition p, subgroup j, feature
        x_grp = x_d[:].rearrange("(g j p) f -> g p j f", g=NGROUP, j=4)

        def load_x(g):
            xg4 = xpool.tile([128, 4, D], F32R, tag="xg", name="xg4")
            nc.sync.dma_start(xg4[:], x_grp[g])
            return xg4

        def group_matvec(g, xg4):
            """cur for batches [512g, 512(g+1)) -> cur_cols[:, 4g:4g+4]."""
            acc = psacc.tile([128, 4], F32, tag="acc")
            nc.vector.memset(acc[:, :], 0.0)
            xts = []
            for ci, (c0, cl) in enumerate(CHUNKS):
                xt_ps = psxt.tile([128, GROUP], F32R, tag="xt", name="xt_ps")
                for j in range(4):
                    nc.tensor.transpose(
                        xt_ps[:cl, j * 128:(j + 1) * 128],
                        xg4[:, j, c0:c0 + cl],
                        id_t[:],
                    )
                xt_sb = xtpool.tile([128, GROUP], F32R, tag="xtsb", name="xt_sb")
                copy(xt_sb[:cl, :], xt_ps[:cl, :])
                xts.append(xt_sb)
            for ci, (c0, cl) in enumerate(CHUNKS):
                for j in range(4):
                    nc.tensor.matmul(
                        acc[:, j:j + 1],
                        xts[ci][:cl, j * 128:(j + 1) * 128].bitcast(F32),
                        w_t[:cl, ci:ci + 1],
                        start=False,
                        stop=(ci == len(CHUNKS) - 1),
                    )
            nc.vector.tensor_copy(cur_cols[:, 4 * g:4 * g + 4], acc[:, :])

        def group_rowform(g):
            """Transpose group g's cur columns into a [1,512] SBUF row.

            Four single-column PE transposes target disjoint 128-wide spans
            of one PSUM bank; the first uses start=True (resets the bank),
            the rest accumulate, avoiding the whole-bank reset clobber.
            """
            row_ps = psct.tile([1, GROUP], F32, tag="ct")
            for c in range(4):
                nc.tensor.matmul(
                    row_ps[0:1, c * 128:(c + 1) * 128],
                    cur_cols[:, 4 * g + c:4 * g + c + 1],
                    id_t[:].bitcast(F32),
                    start=(c == 0), stop=(c == 3), is_transpose=True)
            cur_row = rowpool.tile([1, GROUP], F32, tag="row")
            nc.scalar.copy(cur_row[:, :], row_ps[:, :])
            return cur_row

        def group_tail(g, cur_row):
            """Closed-form mem for group g from its cur row."""
            bc_ps = psbc.tile([128, GROUP], F32, tag="bc")
            nc.tensor.matmul(bc_ps[:, :], ones_t[:, :], cur_row[0:1, :],
                             start=True, stop=True)
            bc_sb = fpool.tile([128, GROUP], F32, tag="bc_sb")
            nc.scalar.copy(bc_sb[:, :], bc_ps[:, :])
            fts = []
            for c in range(2):
                ft = fpool.tile([128, GROUP], F32R, tag=f"f{c}")
                nc.vector.scalar_tensor_tensor(
                    ft[:, :], bc_sb[:, :], thr_t[:, c:c + 1], bc_sb[:, :],
                    ALU.is_gt, ALU.mult)
                fts.append(ft)
            osbs = []
            for tc_i, (t0, tl) in enumerate(TCHUNKS):
                go_ps = psgo.tile([128, GROUP], F32, tag="go")
                for c in range(2):
                    nc.tensor.matmul(
                        go_ps[:tl, :],
                        g_t[:, c * NUM_STEPS + t0:c * NUM_STEPS + t0 + tl],
                        fts[c][:, :],
                        start=(c == 0), stop=(c == 1))
                o_sb = opool.tile([128, GROUP], F32, tag="osb")
                # t-chunk 0 copied on DVE, 1 on ACT; both DMAs issue from
                # ACT with the DVE-copied one last so the ACT queue head
                # never waits on a cross-engine copy.
                if tc_i == 0:
                    nc.vector.tensor_copy(o_sb[:tl, :], go_ps[:tl, :])
                else:
                    nc.scalar.copy(o_sb[:tl, :], go_ps[:tl, :])
                osbs.append((t0, tl, o_sb))
            for t0, tl, o_sb in reversed(osbs):
                nc.scalar.dma_start(
                    mem_d[t0:t0 + tl, g * GROUP:(g + 1) * GROUP],
                    o_sb[:tl, :])

        xq = [load_x(0)]
        # consts issue after the first x tile so the x stream leads the
        # DMA queue; everything consuming them starts later anyway
        nc.sync.dma_start(w_t[:], w_d[:])
        nc.sync.dma_start(id_t[:], id_d[:])
        nc.sync.dma_start(g_t[:], g_d[:])
        nc.sync.dma_start(thr_t[:], thr_d[:])
        nc.sync.dma_start(ones_t[:], ones_d[:])
        pending = None
        for g in range(NGROUP):
            if g + 1 < NGROUP:
                xq.append(load_x(g + 1))
            group_matvec(g, xq.pop(0))
            if pending is not None:
                group_tail(*pending)
            pending = (g, group_rowform(g))
        group_tail(*pending)

    nc.compile()
    return nc


_NC_CACHE = None


def _get_nc():
    global _NC_CACHE
    if _NC_CACHE is None:
        _NC_CACHE = _build()
    return _NC_CACHE


def _round11(a):
    """Round-to-nearest-even at 11 explicit mantissa bits (fp32r grid)."""
    u = np.ascontiguousarray(a, np.float32).view(np.uint32)
    u = (u + 0x800) & 0xFFFFF000
    return u.view(np.float32)


def _host_tables():
    s = np.zeros(NUM_STEPS + 2)
    for k in range(1, NUM_STEPS + 2):
        s[k] = s[k - 1] * BETA + 1.0
    t = np.arange(1, NUM_STEPS + 1)
    R = s[t]

    def pattern(k):
        P = k + 1
        phi = ((t - 1) % P) + 1
        v = s[phi].copy()
        v[phi == P] = 0.0
        return v

    G = np.zeros((NCLASS, NUM_STEPS))
    G[0] = R
    for k in range(1, NCLASS):
        Ak = pattern(k)
        Ak1 = pattern(k + 1) if k + 1 < NCLASS else R
        G[k] = Ak - Ak1
    # gtab layout: [128 partitions, 2 chunks * 255] , class = c*128 + p
    gtab = np.zeros((128, 2 * NUM_STEPS), np.float32)
    for c in range(2):
        gtab[:, c * NUM_STEPS:(c + 1) * NUM_STEPS] = G[c * 128:(c + 1) * 128]
    gtab = _round11(gtab)

    thr = np.zeros((128, 2), np.float32)
    theta = (1.0 / s[1:NCLASS]).astype(np.float32)  # theta_k, k=1..255
    flat = np.concatenate([[np.float32(-3.0e38)], theta])
    thr[:, 0] = flat[0:128]
    thr[:, 1] = flat[128:256]
    return gtab, thr


def _prep_inputs(x, W):
    x = np.ascontiguousarray(np.asarray(x, dtype=np.float32))
    W = np.asarray(W, dtype=np.float32).reshape(-1)
    assert x.shape == (B_FULL, D) and W.shape == (D,)
    wpad = np.zeros(896, np.float32)
    wpad[:D] = W
    wcol = np.ascontiguousarray(wpad.reshape(7, 128).T)
    ident = np.eye(128, dtype=np.float32)
    gtab, thr = _host_tables()
    ones = np.ones((1, 128), np.float32)
    in_maps = [
        {"x": x[d * B_CORE:(d + 1) * B_CORE], "w": wcol, "ident": ident,
         "gtab": gtab, "thr": thr, "ones": ones}
        for d in range(N_CORES)
    ]
    return in_maps


def kernel(x, W, _trace=False, _trace_kwargs=None):
    nc = _get_nc()
    in_maps = _prep_inputs(x, W)
    res = run_bass_kernel_spmd(nc, in_maps, list(range(N_CORES)),
                               trace=_trace, **(_trace_kwargs or {}))
    mem = np.concatenate([res.results[d]["mem"] for d in range(N_CORES)], axis=1)
    mem_rec = mem.reshape(NUM_STEPS, B_FULL, 1)
    spk_rec = (mem_rec > np.float32(THRESHOLD)).astype(np.float32)
    if _trace:
        return (spk_rec, mem_rec), res
    return spk_rec, mem_rec


# revision 19
# speedup vs baseline: 1.0160x; 1.0160x over previous
"""Trainium2 Bass kernel for nn_Net_18906446037087 (snntorch Leaky SNN layer).

Reference semantics (per batch element, 255 steps, f32):
    cur = x @ W.T                         # [B, 1]
    m_0 = 0
    m_{t+1} = (0.95*m_t + cur) * (m_t <= 1)
    spk_{t+1} = (m_{t+1} > 1)
Outputs: (spk_rec, mem_rec), each [255, B, 1] f32.

Sharding: pure data parallel over batch across 8 cores (B=65536 -> 8192/core).

Closed form: the trajectory is periodic in t. With s[k] = (1-b^k)/(1-b),
an element first spikes at step K iff cur > 1/s[K]; then mem repeats the
pattern A_K[t] = s[((t-1) mod (K+1)) + 1] (0 at the reset slot); elements
with cur <= 1/s[255] follow the pure ramp R[t] = s[t]. So
    mem[t, b] = cur_b * (R[t] + sum_{k>=K(b)} (A_k - A_{k+1})[t])
which is one matmul  mem = G^T @ F  with
    G[0] = R, G[k] = A_k - A_{k+1} (A_256 := R)     (host-precomputed)
    F[k, b] = cur_b * [cur_b > theta_k], theta_0 = -inf, theta_k = 1/s[k].
spk is derived on host as mem > 1.0 (exact).

Numerics: the matvec and the G@F matmul run in fp32r (PE rounds operands
to 11 explicit mantissa bits, RNE — measured on HW); end-to-end rel err
~2e-3 vs the 2e-2 gate (validated against the axon-backend oracle).

Per-core pipeline (B_CORE=8192 = 16 groups of 512 = 64 subgroups of 128):
  per group g:
   - one DMA loads x rows as [128, 4, 784] (subgroup-major, contiguous);
   - PE transposes 7 K-chunks (fp32r, 1.5 cyc/row) into PSUM; DVE/ACT
     evacuate to SBUF; 28 matmuls with xT *stationary* and W moving
     (out [128,1]/subgroup, PSUM bank pre-zeroed + start=False because a
     start=True matmul resets its whole PSUM bank) -> cur columns;
   - cur [128,4] is PE-transposed to rows and bounced 2KB through DRAM
     to a [1,512] row (ACT-issued DMAs);
   - one group later (hiding the bounce), PE broadcasts the row to
     [128,512] via a ones outer product, DVE stt builds F (2 class
     chunks, fp32r), two accumulated fp32r matmuls per 128-step slab
     produce mem[t,b] in [t-partition, batch-free] layout, DVE/ACT
     evacuate, ACT issues contiguous 2KB-per-partition writes.
DMA issue queues: x loads alone on SP (so nothing blocks them); all
tail DMAs on ACT, each emitted right after its producing copy.

TimelineSim budget per core: DMA ~96us busy (floor: x in 71.4 + mem out
23.2), PE ~65, DVE ~75, ACT ~55.
"""
import sys
if "/opt/trn_rl_repo" not in sys.path:
    sys.path.insert(0, "/opt/trn_rl_repo")

import numpy as np
from contextlib import ExitStack

import concourse.bass as bass
import concourse.bacc as bacc
import concourse.mybir as mybir
import concourse.tile as tile
from concourse.bass_utils import run_bass_kernel_spmd

F32 = mybir.dt.float32
F32R = mybir.dt.float32r
ALU = mybir.AluOpType

N_CORES = 8
B_FULL = 65536
B_CORE = B_FULL // N_CORES          # 8192
D = 784
NUM_STEPS = 255
BETA = 0.95
THRESHOLD = 1.0

GROUP = 512                          # batch per group
NGROUP = B_CORE // GROUP             # 16
CHUNKS = [(0, 128), (128, 128), (256, 128), (384, 128), (512, 128), (640, 128), (768, 16)]
NCLASS = 256                         # class 0 = ramp; class k = first spike at k
TCHUNKS = [(0, 128), (128, 127)]     # step slabs (255 rows)


def _build():
    nc = bacc.Bacc("TRN2", target_bir_lowering=False, debug=False,
                   num_devices=N_CORES)
    x_d = nc.dram_tensor("x", [B_CORE, D], F32R, kind="ExternalInput")
    w_d = nc.dram_tensor("w", [128, 7], F32, kind="ExternalInput")
    id_d = nc.dram_tensor("ident", [128, 128], F32R, kind="ExternalInput")
    g_d = nc.dram_tensor("gtab", [128, 2 * NUM_STEPS], F32R, kind="ExternalInput")
    thr_d = nc.dram_tensor("thr", [128, 2], F32, kind="ExternalInput")
    ones_d = nc.dram_tensor("ones", [1, 128], F32R, kind="ExternalInput")
    mem_d = nc.dram_tensor("mem", [NUM_STEPS, B_CORE], F32, kind="ExternalOutput")

    with tile.TileContext(nc) as tc, ExitStack() as ctx:
        xpool = ctx.enter_context(tc.tile_pool(name="xpool", bufs=5))
        xtpool = ctx.enter_context(tc.tile_pool(name="xtpool", bufs=4))
        rowpool = ctx.enter_context(tc.tile_pool(name="rowpool", bufs=2))
        fpool = ctx.enter_context(tc.tile_pool(name="fpool", bufs=2))
        opool = ctx.enter_context(tc.tile_pool(name="opool", bufs=4))
        const = ctx.enter_context(tc.tile_pool(name="const", bufs=1))
        psxt = ctx.enter_context(tc.tile_pool(name="psxt", bufs=3, space="PSUM"))
        psacc = ctx.enter_context(tc.tile_pool(name="psacc", bufs=1, space="PSUM"))
        psct = ctx.enter_context(tc.tile_pool(name="psct", bufs=1, space="PSUM"))
        psbc = ctx.enter_context(tc.tile_pool(name="psbc", bufs=1, space="PSUM"))
        psgo = ctx.enter_context(tc.tile_pool(name="psgo", bufs=2, space="PSUM"))

        w_t = const.tile([128, 7], F32)
        id_t = const.tile([128, 128], F32R)
        g_t = const.tile([128, 2 * NUM_STEPS], F32R)
        thr_t = const.tile([128, 2], F32)
        ones_t = const.tile([1, 128], F32R)

        cur_cols = const.tile([128, NGROUP * 4], F32, name="cur_cols")

        copy_engines = [nc.vector.tensor_copy, nc.scalar.copy]
        copy_idx = [0]

        def copy(out, in_):
            eng = copy_engines[copy_idx[0] % len(copy_engines)]
            copy_idx[0] += 1
            eng(out, in_)

        # x rows for group g as [128, 4, D]: partition p, subgroup j, feature
        x_grp = x_d[:].rearrange("(g j p) f -> g p j f", g=NGROUP, j=4)

        def load_x(g):
            xg4 = xpool.tile([128, 4, D], F32R, tag="xg", name="xg4")
            nc.sync.dma_start(xg4[:], x_grp[g])
            return xg4

        def group_matvec(g, xg4):
            """cur for batches [512g, 512(g+1)) -> cur_cols[:, 4g:4g+4]."""
            acc = psacc.tile([128, 4], F32, tag="acc")
            nc.vector.memset(acc[:, :], 0.0)
            xts = []
            for ci, (c0, cl) in enumerate(CHUNKS):
                xt_ps = psxt.tile([128, GROUP], F32R, tag="xt", name="xt_ps")
                for j in range(4):
                    nc.tensor.transpose(
                        xt_ps[:cl, j * 128:(j + 1) * 128],
                        xg4[:, j, c0:c0 + cl],
                        id_t[:],
                    )
                xt_sb = xtpool.tile([128, GROUP], F32R, tag="xtsb", name="xt_sb")
                copy(xt_sb[:cl, :], xt_ps[:cl, :])
                xts.append(xt_sb)
            for ci, (c0, cl) in enumerate(CHUNKS):
                for j in range(4):
                    nc.tensor.matmul(
                        acc[:, j:j + 1],
                        xts[ci][:cl, j * 128:(j + 1) * 128].bitcast(F32),
                        w_t[:cl, ci:ci + 1],
                        start=False,
                        stop=(ci == len(CHUNKS) - 1),
                    )
            nc.vector.tensor_copy(cur_cols[:, 4 * g:4 * g + 4], acc[:, :])

        def group_rowform(g):
            """Transpose group g's cur columns into a [1,512] SBUF row.

            Four single-column PE transposes target disjoint 128-wide spans
            of one PSUM bank; the first uses start=True (resets the bank),
            the rest accumulate, avoiding the whole-bank reset clobber.
            """
            row_ps = psct.tile([1, GROUP], F32, tag="ct")
            for c in range(4):
                nc.tensor.matmul(
                    row_ps[0:1, c * 128:(c + 1) * 128],
                    cur_cols[:, 4 * g + c:4 * g + c + 1],
                    id_t[:].bitcast(F32),
                    start=(c == 0), stop=(c == 3), is_transpose=True)
            cur_row = rowpool.tile([1, GROUP], F32R, tag="row")
            nc.vector.tensor_copy(cur_row[:, :], row_ps[:, :])
            return cur_row

        def group_tail(g, cur_row):
            """Closed-form mem for group g from its cur row."""
            bc_ps = psbc.tile([128, GROUP], F32, tag="bc")
            nc.tensor.matmul(bc_ps[:, :], ones_t[:, :], cur_row[0:1, :],
                             start=True, stop=True)
            bc_sb = fpool.tile([128, GROUP], F32, tag="bc_sb")
            nc.scalar.copy(bc_sb[:, :], bc_ps[:, :])
            fts = []
            for c in range(2):
                ft = fpool.tile([128, GROUP], F32R, tag=f"f{c}")
                nc.vector.scalar_tensor_tensor(
                    ft[:, :], bc_sb[:, :], thr_t[:, c:c + 1], bc_sb[:, :],
                    ALU.is_gt, ALU.mult)
                fts.append(ft)
            osbs = []
            for tc_i, (t0, tl) in enumerate(TCHUNKS):
                go_ps = psgo.tile([128, GROUP], F32, tag="go")
                for c in range(2):
                    nc.tensor.matmul(
                        go_ps[:tl, :],
                        g_t[:, c * NUM_STEPS + t0:c * NUM_STEPS + t0 + tl],
                        fts[c][:, :],
                        start=(c == 0), stop=(c == 1))
                o_sb = opool.tile([128, GROUP], F32, tag="osb")
                # t-chunk 0 copied on DVE, 1 on ACT; both DMAs issue from
                # ACT with the DVE-copied one last so the ACT queue head
                # never waits on a cross-engine copy.
                if tc_i == 0:
                    nc.vector.tensor_copy(o_sb[:tl, :], go_ps[:tl, :])
                else:
                    nc.scalar.copy(o_sb[:tl, :], go_ps[:tl, :])
                osbs.append((t0, tl, o_sb))
            for t0, tl, o_sb in reversed(osbs):
                nc.scalar.dma_start(
                    mem_d[t0:t0 + tl, g * GROUP:(g + 1) * GROUP],
                    o_sb[:tl, :])

        xq = [load_x(0)]
        # consts issue after the first x tile so the x stream leads the
        # DMA queue; everything consuming them starts later anyway
        nc.sync.dma_start(w_t[:], w_d[:])
        nc.sync.dma_start(id_t[:], id_d[:])
        nc.sync.dma_start(g_t[:], g_d[:])
        nc.sync.dma_start(thr_t[:], thr_d[:])
        nc.sync.dma_start(ones_t[:], ones_d[:])
        pending = None
        for g in range(NGROUP):
            if g + 1 < NGROUP:
                xq.append(load_x(g + 1))
            group_matvec(g, xq.pop(0))
            if pending is not None:
                group_tail(*pending)
            pending = (g, group_rowform(g))
        group_tail(*pending)

    nc.compile()
    return nc


_NC_CACHE = None


def _get_nc():
    global _NC_CACHE
    if _NC_CACHE is None:
        _NC_CACHE = _build()
    return _NC_CACHE


def _round11(a):
    """Round-to-nearest-even at 11 explicit mantissa bits (fp32r grid)."""
    u = np.ascontiguousarray(a, np.float32).view(np.uint32)
    u = (u + 0x800) & 0xFFFFF000
    return u.view(np.float32)


def _host_tables():
    s = np.zeros(NUM_STEPS + 2)
    for k in range(1, NUM_STEPS + 2):
        s[k] = s[k - 1] * BETA + 1.0
    t = np.arange(1, NUM_STEPS + 1)
    R = s[t]

    def pattern(k):
        P = k + 1
        phi = ((t - 1) % P) + 1
        v = s[phi].copy()
        v[phi == P] = 0.0
        return v

    G = np.zeros((NCLASS, NUM_STEPS))
    G[0] = R
    for k in range(1, NCLASS):
        Ak = pattern(k)
        Ak1 = pattern(k + 1) if k + 1 < NCLASS else R
        G[k] = Ak - Ak1
    # gtab layout: [128 partitions, 2 chunks * 255] , class = c*128 + p
    gtab = np.zeros((128, 2 * NUM_STEPS), np.float32)
    for c in range(2):
        gtab[:, c * NUM_STEPS:(c + 1) * NUM_STEPS] = G[c * 128:(c + 1) * 128]
    gtab = _round11(gtab)

    thr = np.zeros((128, 2), np.float32)
    theta = (1.0 / s[1:NCLASS]).astype(np.float32)  # theta_k, k=1..255
    flat = np.concatenate([[np.float32(-3.0e38)], theta])
    thr[:, 0] = flat[0:128]
    thr[:, 1] = flat[128:256]
    return gtab, thr


def _prep_inputs(x, W):
    x = np.ascontiguousarray(np.asarray(x, dtype=np.float32))
    W = np.asarray(W, dtype=np.float32).reshape(-1)
    assert x.shape == (B_FULL, D) and W.shape == (D,)
    wpad = np.zeros(896, np.float32)
    wpad[:D] = W
    wcol = np.ascontiguousarray(wpad.reshape(7, 128).T)
    ident = np.eye(128, dtype=np.float32)
    gtab, thr = _host_tables()
    ones = np.ones((1, 128), np.float32)
    in_maps = [
        {"x": x[d * B_CORE:(d + 1) * B_CORE], "w": wcol, "ident": ident,
         "gtab": gtab, "thr": thr, "ones": ones}
        for d in range(N_CORES)
    ]
    return in_maps


def kernel(x, W, _trace=False, _trace_kwargs=None):
    nc = _get_nc()
    in_maps = _prep_inputs(x, W)
    res = run_bass_kernel_spmd(nc, in_maps, list(range(N_CORES)),
                               trace=_trace, **(_trace_kwargs or {}))
    mem = np.concatenate([res.results[d]["mem"] for d in range(N_CORES)], axis=1)
    mem_rec = mem.reshape(NUM_STEPS, B_FULL, 1)
    spk_rec = (mem_rec > np.float32(THRESHOLD)).astype(np.float32)
    if _trace:
        return (spk_rec, mem_rec), res
    return spk_rec, mem_rec
